# revision 31
# baseline (speedup 1.0000x reference)
"""Fused int8 dequant -> causal mask -> softmax -> int8 requant on 8 TRN2 cores.

Problem: x_q [B=4, H=16, S=1024, S] int8, per-(head,row) scales sx/so [H*S] f32.
  out = int8(clip(round(softmax(causal_mask(x_q * sx)) / so), -128, 127))

Sharding: 2 heads per core (data parallel over 64 independent (b, h) planes;
grouping by head lets the 4 batches of one head share per-partition scale
vectors, so the exp runs as one instruction per (h, row-tile)).

Rows live on partitions; softmax runs along the free dim. For each (h, t)
row-tile of 128 rows, only cols [0, W=(t+1)*128) can be nonzero (causal), so
only those are moved. Host packs x premasked (strict upper zeroed) in a
per-(h,t) [128, B*W] layout so every DMA is 128 contiguous descriptors.

Engine budget per core (measured cost model: ACT 380ns + 0.833ns/elt,
DVE ~160-220ns + 0.52ns/elt 2x / 1.04 reduce, ACT accum readout 279ns):
  - exp must run on ACT (~31us of element time); requant (fp16 -> int8
    round+sat) runs on DVE at 2x_2P (~19us + per-instr overhead).
  - row sums are the swing work: ACT's accumulator gets them "free" but
    forces per-(b,t) exp granularity (+3x380 +4x279 ns per tile); DVE pays
    1.04ns/elt reduce, cut to ~0.65/elt by pairwise fp16 fold adds at 2x.
    Tiles t in ACT_SUM_T use the ACT path; the rest fold+reduce on DVE,
    balancing the two engines at ~50us each.
  - masked diag-upper cleanup moved to the HOST (out *= tril after unpack),
    removing 16 DVE tensor_tensor ops; sums still corrected by the
    compile-time (127-p) constant since premasked x contributes exp(0)=1.
  - smalls fused: one tensor_scalar (sum-corr)*so [two per-partition
    scalars] + one approx-reciprocal custom op per tile.
  - DMA triggers moved off the Sync queue: x-in on the (idle) TensorE
    queue, y-out on the GpSimd queue.
"""

import contextlib
import ctypes
import os
import sys
import types
from contextlib import ExitStack

import numpy as np

import concourse.bacc as bacc
import concourse.bass as bass
import concourse.tile as tile
from concourse import mybir
from concourse.bass_utils import run_bass_kernel_spmd

B, H, S = 4, 16, 1024
NCORES = 8
HPC = H // NCORES  # heads per core
P = 128
NT = S // P  # row tiles per plane
AF = mybir.ActivationFunctionType
ALU = mybir.AluOpType

# packed block offsets: block (h, t) holds [P, B*W] int8, W = (t+1)*P
_BLK = [[None] * NT for _ in range(HPC)]
_off = 0
for _h in range(HPC):
    for _t in range(NT):
        _W = (_t + 1) * P
        _BLK[_h][_t] = (_off, _W)
        _off += P * B * _W
TOTAL = _off  # per-core packed bytes (4718592)

_AXON_SO = "/opt/axon/libaxon_pjrt.so"


def _ensure_ntff_hook():
    """This image's antenv lacks axon_hooks; provide it so trace=True works."""
    if "antenv.axon_hooks" in sys.modules:
        return
    import antenv

    mod = types.ModuleType("antenv.axon_hooks")
    state = {"hook": None}
    mod.set_axon_ntff_profile_hook = lambda h: state.__setitem__("hook", h)
    mod.get_axon_ntff_profile_hook = lambda: state["hook"]
    sys.modules["antenv.axon_hooks"] = mod
    antenv.axon_hooks = mod

    if not os.path.exists(_AXON_SO):
        return
    lib = ctypes.CDLL(_AXON_SO)
    if not hasattr(lib, "axon_start_nrt_profile"):
        return
    lib.axon_start_nrt_profile.argtypes = [ctypes.POINTER(ctypes.c_int64), ctypes.c_size_t]
    lib.axon_start_nrt_profile.restype = ctypes.c_int64
    lib.axon_stop_nrt_profile.argtypes = [ctypes.c_char_p]
    lib.axon_stop_nrt_profile.restype = ctypes.c_int64

    @contextlib.contextmanager
    def _hook(output_dir, device_ids):
        import jax

        jax.devices()
        if device_ids:
            ids = (ctypes.c_int64 * len(device_ids))(*device_ids)
            rc = lib.axon_start_nrt_profile(ids, len(device_ids))
        else:
            rc = lib.axon_start_nrt_profile(None, 0)
        if rc != 0:
            raise RuntimeError(f"axon_start_nrt_profile rc={rc}")
        try:
            yield
        finally:
            n = lib.axon_stop_nrt_profile(str(output_dir).encode())
            print(f"profile: {n} file(s) written to {output_dir}", file=sys.stderr)

    mod.set_axon_ntff_profile_hook(_hook)


_cached_nc = None

# tiles whose row sums ride the ACT accumulator (per-b exp); the rest
# fold+reduce on DVE. Balance point measured on HW.
ACT_SUM_T = frozenset(
    int(c) for c in os.environ.get("ACT_SUM_T", "67")
)
# per-h processing order as PAIRS: each pair shares one [P, 2B] sums buffer
# and one fused smalls chain (scalar_tensor_tensor + approx reciprocal).
# First tile small (fast first DMA -> early first exp); last pair small
# (short drain chain); ACT-heavy (5,6,7) interleaved with DVE-heavy (0-4).


def _parse_pairs(s):
    return [(int(a), int(b)) for a, b in (p.split(",") for p in s.split())]


# DVE-class (batched-exp) tile FIRST in each mixed pair so DVE's folds can
# start while ACT grinds the per-b tile.
PAIRS0 = _parse_pairs(os.environ.get("PAIRS0", "1,0 2,7 5,6 3,4"))
PAIRS1 = _parse_pairs(os.environ.get("PAIRS1", "2,7 3,4 5,1 6,0"))
# software-pipelined emit order: "eT" = exp(+sums) of tile T, "fK" = finish
# pair K (smalls, requant, store). Each pair's DVE-class exp is hoisted ahead
# of the previous pair's ACT-class per-b exps so DVE always has fold work.
SCHED0 = os.environ.get("SCHED0", "e1 e0 f0 e2 e3 e5 e4 e7 f1 f3 e6 f2").split()
SCHED1 = os.environ.get("SCHED1", "e2 e5 e7 f0 e3 e4 f1 e1 f2 e0 e6 f3").split()
# tiles whose sums run on the GpSimd engine via per-b tensor_scalar+accum.
# Empty: the Pool engine rejects the accum opcode (codegen engine check).
GPS_SUM_T = frozenset(int(c) for c in os.environ.get("GPS_SUM_T", ""))
# DVE-sum tiles using one fused fold+accum (tensor_tensor_reduce) per b.
# Empty: InstTensorTensorReduce wedges the device (NRT_EXEC_UNIT_UNRECOVERABLE).
TTR_SUM_T = frozenset(int(c) for c in os.environ.get("TTR_SUM_T", ""))
# tiles requanted by per-b in-place tensor_scalar (4x mode, fp16 out)
# instead of one 2x tensor_tensor; wins for large W
REQUANT_TS_T = frozenset(int(c) for c in os.environ.get("REQUANT_TS_T", "567"))


def _v3(t2d, outer_stride, n_outer, n_inner, offset=0):
    """[P, outer, inner] strided view of a 2D [P, V] tile AP."""
    return bass.AP(
        tensor=t2d.tensor,
        offset=t2d.offset + offset,
        ap=[t2d.ap[0], [outer_stride, n_outer], [1, n_inner]],
    )


def _build_bass(compile=True):
    nc = bacc.Bacc("TRN2", target_bir_lowering=False, debug=False,
                   num_devices=NCORES)
    x = nc.declare_dram_parameter("x", [TOTAL], mybir.dt.int8, isOutput=False)
    sx = nc.declare_dram_parameter("sx", [P, HPC * NT], mybir.dt.float32, isOutput=False)
    # so replicated x4 (per b), laid out in processing order per pair
    sob = nc.declare_dram_parameter("sob", [P, HPC * NT * B], mybir.dt.float32,
                                    isOutput=False)
    corr = nc.declare_dram_parameter("corr", [P, 1], mybir.dt.float32, isOutput=False)
    y = nc.declare_dram_parameter("y", [TOTAL], mybir.dt.int8, isOutput=True)

    with ExitStack() as ctx:
        tc = ctx.enter_context(tile.TileContext(nc))
        singles = ctx.enter_context(tc.tile_pool(name="singles", bufs=1))
        xpool = ctx.enter_context(tc.tile_pool(name="xp", bufs=8))
        epool = ctx.enter_context(tc.tile_pool(name="ep", bufs=9))
        spool = ctx.enter_context(tc.tile_pool(name="sp", bufs=4))
        smalls = ctx.enter_context(tc.tile_pool(name="sm", bufs=8))

        sxt = singles.tile([P, HPC * NT], mybir.dt.float32)
        nc.sync.dma_start(sxt[:], sx[:])
        corrt = singles.tile([P, 1], mybir.dt.float32)
        nc.sync.dma_start(corrt[:], corr[:])
        # tiny exp depending only on the (early, small) corr DMA: pulls the
        # ~1.3us ACT_TABLE_LOAD off the first-real-exp critical path
        dummy = singles.tile([P, 1], mybir.dt.float16)
        nc.scalar.activation(dummy[:], corrt[:], AF.Exp, bias=0.0, scale=0.0)
        # sob is first needed ~10us in; defer its DMA trigger behind the
        # first pair's x DMAs on the sync queue
        sobt = singles.tile([P, HPC * NT * B], mybir.dt.float32)

        def _exp_and_sums(h, t, sums, sumoff):
            """DMA in + exp (+ row sums via ACT accum or DVE fold+reduce).
            Returns the et tile and W."""
            off, W = _BLK[h][t]
            col = h * NT + t
            V = B * W
            xt = xpool.tile([P, V], mybir.dt.int8, tag="xt")
            nc.sync.dma_start(
                xt[:], x[off:off + P * V].rearrange("(p n) -> p n", p=P))
            et = epool.tile([P, V], mybir.dt.float16, tag="et")
            if t in ACT_SUM_T:
                # per-b exp with free row sums from the ACT accumulator
                for b in range(B):
                    nc.scalar.activation(et[:, b * W:(b + 1) * W],
                                         xt[:, b * W:(b + 1) * W],
                                         AF.Exp, bias=0.0,
                                         scale=sxt[:, col:col + 1],
                                         accum_out=sums[:, sumoff + b:sumoff + b + 1])
            elif t in GPS_SUM_T:
                # batched exp; per-b sums on the idle GpSimd engine
                nc.scalar.activation(et[:], xt[:], AF.Exp, bias=0.0,
                                     scale=sxt[:, col:col + 1])
                scr = spool.tile([P, B * W], mybir.dt.float16, tag="gscr")
                for b in range(B):
                    nc.gpsimd.tensor_scalar(
                        scr[:, b * W:(b + 1) * W], et[:, b * W:(b + 1) * W],
                        1.0, None, ALU.mult, ALU.add,
                        accum_out=sums[:, sumoff + b:sumoff + b + 1])
            elif t in TTR_SUM_T:
                # batched exp; per-b fused half-fold + accumulator reduce
                nc.scalar.activation(et[:], xt[:], AF.Exp, bias=0.0,
                                     scale=sxt[:, col:col + 1])
                hw2 = W // 2
                scr = spool.tile([P, B * hw2], mybir.dt.float16, tag="sttr")
                for b in range(B):
                    nc.vector.tensor_tensor_reduce(
                        scr[:, b * hw2:(b + 1) * hw2],
                        et[:, b * W:b * W + hw2],
                        et[:, b * W + hw2:(b + 1) * W],
                        1.0, 0.0, ALU.add, ALU.add,
                        accum_out=sums[:, sumoff + b:sumoff + b + 1])
            else:
                # one batched exp; sums via fp16 pairwise folds (2x TT)
                # and a final 3D tensor_reduce on DVE
                nc.scalar.activation(et[:], xt[:], AF.Exp, bias=0.0,
                                     scale=sxt[:, col:col + 1])
                folds = 0 if t == 0 else (2 if t <= 3 else 3)
                cur, curw = et[:], W
                for i in range(folds):
                    nw = curw // 2
                    s = spool.tile([P, B * nw], mybir.dt.float16, tag=f"s{i}")
                    nc.vector.tensor_tensor(
                        _v3(s[:], nw, B, nw),
                        _v3(cur, curw, B, nw),
                        _v3(cur, curw, B, nw, offset=nw),
                        ALU.add)
                    cur, curw = s[:], nw
                nc.vector.tensor_reduce(
                    sums[:, sumoff:sumoff + B], _v3(cur, curw, B, curw),
                    mybir.AxisListType.X, ALU.add)
            return et, W

        def _requant_out(h, t, et, rt, rt2, rtoff):
            """In-place et *= r on DVE (2x TT with pair-duplicated r for
            small tiles; per-b 4x tensor_scalar for large), then one SWDGE
            cast-DMA fp16 -> int8 (hardware round-to-nearest-even +
            saturate, verified) straight to DRAM."""
            off, W = _BLK[h][t]
            V = B * W
            yv = y[off:off + P * V].rearrange("(p n) -> p n", p=P)
            if t in REQUANT_TS_T:
                # per-b requant; each quarter ships as soon as it's scaled,
                # so the final DMA drain is a quarter-tile deep
                for b in range(B):
                    nc.vector.tensor_scalar(
                        et[:, b * W:(b + 1) * W], et[:, b * W:(b + 1) * W],
                        rt[:, rtoff + b:rtoff + b + 1], None, ALU.mult)
                    nc.gpsimd.dma_start(yv[:, b * W:(b + 1) * W],
                                        et[:, b * W:(b + 1) * W])
            else:
                ev = bass.AP(tensor=et.tensor, offset=et.offset,
                             ap=[et.ap[0], [W, B], [2, W // 2], [1, 2]])
                rv = bass.AP(tensor=rt2.tensor, offset=rt2.offset + 2 * rtoff,
                             ap=[rt2.ap[0], [2, B], [0, W // 2], [1, 2]])
                nc.vector.tensor_tensor(ev, ev, rv, ALU.mult)
                nc.gpsimd.dma_start(yv, et[:])

        nexp = 0
        for h, pairs, sched in ((0, PAIRS0, SCHED0), (1, PAIRS1, SCHED1)):
            state = {}  # pair idx -> {"sums": tile, t: et}
            t2pair = {}
            for k, (ta, tb) in enumerate(pairs):
                t2pair[ta] = (k, 0)
                t2pair[tb] = (k, B)
            for tok in sched:
                if tok[0] == "e":
                    t = int(tok[1:])
                    k, sumoff = t2pair[t]
                    st = state.setdefault(k, {})
                    if "sums" not in st:
                        st["sums"] = smalls.tile([P, 2 * B], mybir.dt.float32,
                                                 tag="sums", name=f"sums_{h}_{k}")
                    st[t], _ = _exp_and_sums(h, t, st["sums"], sumoff)
                    nexp += 1
                    if nexp == 2:
                        # sob first needed by the first fin; its DMA trigger
                        # queues behind the first two x DMAs
                        nc.sync.dma_start(sobt[:], sob[:])
                else:
                    k = int(tok[1:])
                    ta, tb = pairs[k]
                    j = h * len(PAIRS0) + k  # sob slice: pair-order layout
                    st = state.pop(k)
                    sums = st["sums"]
                    # r = 1/((sum - corr) * so) for both tiles in one fused
                    # (sums - corr) * sob chain + approx reciprocal on [P, 8]
                    rinv = smalls.tile([P, 2 * B], mybir.dt.float32, tag="rinv")
                    nc.vector.scalar_tensor_tensor(
                        rinv[:], sums[:], corrt[:],
                        sobt[:, j * 2 * B:(j + 1) * 2 * B],
                        ALU.subtract, ALU.mult)
                    rt = smalls.tile([P, 2 * B], mybir.dt.float32, tag="rt")
                    nc.vector.reciprocal_approx_fast(rt[:], rinv[:])
                    # rt2 = rt duplicated into adjacent fp16 pairs (stride-0
                    # read dim): [r0,r0,r1,r1,...]
                    rt2 = smalls.tile([P, 4 * B], mybir.dt.float16, tag="rt2")
                    nc.vector.tensor_copy(
                        rt2[:],
                        bass.AP(tensor=rt.tensor, offset=rt.offset,
                                ap=[rt.ap[0], [1, 2 * B], [0, 2]]))
                    _requant_out(h, ta, st[ta], rt, rt2, 0)
                    _requant_out(h, tb, st[tb], rt, rt2, B)
    if compile:
        nc.compile()
    return nc


_tril_mask = None


def _host_prep(x_q, scale_x, scale_out):
    global _tril_mask
    x_q = np.asarray(x_q)
    assert x_q.dtype == np.int8, x_q.dtype
    scale_x = np.asarray(scale_x, dtype=np.float32).reshape(H, S)
    scale_out = np.asarray(scale_out, dtype=np.float32).reshape(H, S)

    if _tril_mask is None:
        _tril_mask = np.tril(np.ones((S, S), dtype=np.int8))
    x_pm = x_q * _tril_mask  # zero the strict upper triangle

    # [P, H, NT]: sxr[p, h, t] = scale_x[h, t*128 + p]
    sxr = scale_x.reshape(H, NT, P).transpose(2, 0, 1)
    sor = scale_out.reshape(H, NT, P).transpose(2, 0, 1)

    corr = (127 - np.arange(P)).astype(np.float32).reshape(P, 1)

    in_maps = []
    for c in range(NCORES):
        xc = np.empty(TOTAL, np.int8)
        for h in range(HPC):
            hg = c * HPC + h
            for t in range(NT):
                off, W = _BLK[h][t]
                # [B, P, W] -> [P, B, W] flattened
                blk = x_pm[:, hg, t * P:(t + 1) * P, 0:W].transpose(1, 0, 2)
                xc[off:off + P * B * W] = blk.reshape(-1)
        hs = slice(c * HPC, (c + 1) * HPC)
        sxc = np.ascontiguousarray(sxr[:, hs].reshape(P, HPC * NT))
        # sob: so replicated x4 per b, in pair processing order
        sobc = np.empty((P, HPC * NT * B), np.float32)
        i = 0
        for h, pairs in ((0, PAIRS0), (1, PAIRS1)):
            for ta, tb in pairs:
                for t in (ta, tb):
                    sobc[:, i * B:(i + 1) * B] = sor[:, c * HPC + h, t:t + 1]
                    i += 1
        in_maps.append({"x": xc, "sx": sxc, "sob": sobc, "corr": corr})
    return in_maps


def _host_unpack(results):
    global _tril_mask
    out = np.zeros((B, H, S, S), np.int8)
    for c in range(NCORES):
        yc = np.asarray(results[c]["y"])
        for h in range(HPC):
            hg = c * HPC + h
            for t in range(NT):
                off, W = _BLK[h][t]
                blk = yc[off:off + P * B * W].reshape(P, B, W).transpose(1, 0, 2)
                out[:, hg, t * P:(t + 1) * P, 0:W] = blk
    if _tril_mask is None:
        _tril_mask = np.tril(np.ones((S, S), dtype=np.int8))
    out *= _tril_mask  # device leaves garbage in the diag-block strict upper
    return out


def run(x_q, scale_x, scale_out, trace=False):
    global _cached_nc
    if trace:
        _ensure_ntff_hook()
    if _cached_nc is None:
        _cached_nc = _build_bass()
    in_maps = _host_prep(x_q, scale_x, scale_out)
    res = run_bass_kernel_spmd(_cached_nc, in_maps, core_ids=list(range(NCORES)),
                               trace=trace)
    return _host_unpack(res.results), res


def kernel(x_q, scale_x, scale_out):
    out, _ = run(x_q, scale_x, scale_out,
                 trace=bool(int(os.environ.get("KERNEL_TRACE", "0"))))
    return out
